# revision 8
# baseline (speedup 1.0000x reference)
"""Trainium2 Bass kernel for the Alignment (decomposable-attention) model.

Full inputs in, full outputs out.  Internally: data-parallel over batch
across 8 NeuronCores (4 batch elements per core) for the align/compare
phases; weight-sharded MLP classifier with AllGather(agg) + AllReduce(z2)
collectives.

Precision: the tanh-projection (F) stage runs in true fp32 matmuls (the
softmax selection is chaotic w.r.t. e-matrix errors); the e/attention/
compare stages run in float32r (~13-bit mantissa single-pass matmuls,
measured rel err ~1.5e-4 per dot), which end-to-end gives ~3e-3 relative
error on logits vs the fp32 reference.
"""
import sys

sys.path.insert(0, "/opt/trn_rl_repo")

import numpy as np

import concourse.bacc as bacc
import concourse.tile as tile
import concourse.mybir as mybir

F32 = mybir.dt.float32
F32R = mybir.dt.float32r
AF = mybir.ActivationFunctionType

N_CORES = 8
B_GLOBAL = 32
B = B_GLOBAL // N_CORES  # 4 local batch elements
L = 256
D = 1024
ALIGN = 1024
FF = 2048
FF_SH = FF // N_CORES  # 256: per-core shard of the MLP hidden cols/rows
NCLS = 3

DT = D // 128     # 8 k-tiles over D
AT = ALIGN // 128  # 8 m-tiles over ALIGN
LT = L // 128     # 2 tiles over sequence
KT = 2 * D // 128  # 16 contraction tiles over 2*D for W_G
FT_N = FF // 128   # 16 ff tiles


def build():
    nc = bacc.Bacc("TRN2", target_bir_lowering=False, debug=False,
                   num_devices=N_CORES)

    prem = nc.dram_tensor("premises", [B, L, D], F32, kind="ExternalInput")
    hypo = nc.dram_tensor("hypotheses", [B, L, D], F32, kind="ExternalInput")
    w_f = nc.dram_tensor("W_F", [D, ALIGN], F32, kind="ExternalInput")
    w_g = nc.dram_tensor("W_G", [2 * D, FF], F32, kind="ExternalInput")
    w1s = nc.dram_tensor("W1s", [2 * FF, FF_SH], F32, kind="ExternalInput")
    b1s = nc.dram_tensor("b1s", [FF_SH // 128, 128], F32, kind="ExternalInput")
    w2s = nc.dram_tensor("W2s", [FF_SH, FF], F32, kind="ExternalInput")
    b2r = nc.dram_tensor("b2r", [FF // 128, 128], F32, kind="ExternalInput")
    w3 = nc.dram_tensor("W3", [FF, NCLS], F32, kind="ExternalInput")
    b3 = nc.dram_tensor("b3", [NCLS, 1], F32, kind="ExternalInput")
    out = nc.dram_tensor("logitsT", [NCLS, B_GLOBAL], F32, kind="ExternalOutput")

    idn_dram = nc.inline_tensor(np.eye(128, dtype=np.float32), name="idn")

    with tile.TileContext(nc) as tc:
        with (
            tc.tile_pool(name="const", bufs=1) as cpool,
            tc.tile_pool(name="stream", bufs=3) as spool,
            tc.tile_pool(name="act", bufs=2) as apool,
            tc.tile_pool(name="single", bufs=1) as onepool,
            tc.tile_pool(name="scr", bufs=4) as scrpool,
            tc.tile_pool(name="ps512", bufs=4, space="PSUM") as ps512,
            tc.tile_pool(name="pse", bufs=2, space="PSUM") as pse,
            tc.tile_pool(name="pstp", bufs=2, space="PSUM") as pstp,
            tc.tile_pool(name="dram", bufs=1, space="DRAM") as dpool,
        ):
            # ---- constants ----
            wf_sb = cpool.tile([128, DT, ALIGN], F32, tag="wf")
            nc.sync.dma_start(
                wf_sb[:], w_f.ap().rearrange("(t p) a -> p t a", p=128)
            )
            idn = cpool.tile([128, 128], F32, tag="idn")
            nc.sync.dma_start(idn[:], idn_dram.ap())
            idn_r = idn[:].bitcast(F32R)
            b1_sb = cpool.tile([128, FF_SH // 128], F32, tag="b1")
            nc.sync.dma_start(b1_sb[:], b1s.ap().rearrange("m p -> p m"))
            b2_sb = cpool.tile([128, FF // 128], F32, tag="b2")
            nc.sync.dma_start(b2_sb[:], b2r.ap().rearrange("m p -> p m"))
            b3_sb = cpool.tile([128, 1], F32, tag="b3")
            nc.sync.dma_start(b3_sb[:NCLS, :], b3.ap())
            w3_sb = cpool.tile([128, FT_N, NCLS], F32, tag="w3")
            nc.sync.dma_start(
                w3_sb[:], w3.ap().rearrange("(t p) c -> p t c", p=128)
            )

            agg_sb = onepool.tile([128, 2 * FT_N * B], F32, tag="agg")

            for b in range(B):
                # ---- load this batch element's premise/hypothesis ----
                # phseq cols: [P_i0 | P_i1 | H_j0 | H_j1], each 1024 wide
                phseq = apool.tile([128, 4 * D], F32, tag="phseq")
                for it in range(LT):
                    nc.sync.dma_start(
                        phseq[:, it * D:(it + 1) * D],
                        prem[b, it * 128:(it + 1) * 128, :],
                    )
                    nc.sync.dma_start(
                        phseq[:, (LT + it) * D:(LT + it + 1) * D],
                        hypo[b, it * 128:(it + 1) * 128, :],
                    )
                # f32r copy for the attention-apply (betas/alphas) matmuls
                phseq_r = onepool.tile([128, 4 * D], F32R, tag="phseqr")
                nc.gpsimd.dma_start(
                    phseq_r[:, 0:2 * D].rearrange("p (t d) -> p t d", d=D),
                    prem[b].rearrange("(t p) d -> p t d", p=128),
                )
                nc.gpsimd.dma_start(
                    phseq_r[:, 2 * D:4 * D].rearrange("p (t d) -> p t d", d=D),
                    hypo[b].rearrange("(t p) d -> p t d", p=128),
                )

                # ---- transpose P, H into feature-major PHT ----
                # pht cols: d-tile d (8) * 512 + [P.T cols 0:256 | H.T 256:512]
                pht = apool.tile([128, DT * 2 * L], F32, tag="pht")
                for side in range(2):  # 0: P, 1: H
                    for it in range(LT):
                        src_col = (side * LT + it) * D
                        for d in range(DT):
                            ptp = pstp.tile([128, 128], F32, tag="tp")
                            nc.tensor.transpose(
                                ptp[:],
                                phseq[:, src_col + d * 128: src_col + (d + 1) * 128],
                                idn[:],
                            )
                            nc.vector.tensor_copy(
                                pht[:, d * 2 * L + side * L + it * 128:
                                       d * 2 * L + side * L + (it + 1) * 128],
                                ptp[:],
                            )
                # f32r copy of PHT for the compare (V) stage
                pht_r = onepool.tile([128, DT * 2 * L], F32R, tag="phtr")
                for d in range(DT):
                    nc.vector.tensor_copy(
                        pht_r[:, d * 2 * L:(d + 1) * 2 * L],
                        pht[:, d * 2 * L:(d + 1) * 2 * L],
                    )

                # ---- F stage (fp32): F = tanh([P|H] @ W_F), feature-major ----
                # ft cols: a-tile a (8) * 512 + [Fp 0:256 | Fh 256:512]
                ft = onepool.tile([128, AT * 2 * L], F32R, tag="ft")
                for a in range(AT):
                    psf = ps512.tile([128, 2 * L], F32, tag="mm512")
                    for d in range(DT):
                        nc.tensor.matmul(
                            psf[:],
                            wf_sb[:, d, a * 128:(a + 1) * 128],
                            pht[:, d * 2 * L:(d + 1) * 2 * L],
                            start=(d == 0),
                            stop=(d == DT - 1),
                        )
                    nc.scalar.activation(
                        ft[:, a * 2 * L:(a + 1) * 2 * L], psf[:], AF.Tanh
                    )

                # ---- e + softmax (f32r matmul, fp32 softmax) ----
                exp_sb = onepool.tile([128, LT * L], F32, tag="exp")
                attn_r = onepool.tile([128, LT * L], F32R, tag="attn")
                attnt_r = onepool.tile([128, LT * L], F32R, tag="attnt")
                for it in range(LT):
                    smax = scrpool.tile([128, 3], F32, tag="smax")
                    pe_ = pse.tile([128, L], F32, tag="e")
                    for a in range(AT):
                        nc.tensor.matmul(
                            pe_[:],
                            ft[:, a * 2 * L + it * 128: a * 2 * L + (it + 1) * 128],
                            ft[:, a * 2 * L + L: (a + 1) * 2 * L],
                            start=(a == 0),
                            stop=(a == AT - 1),
                        )
                    nc.vector.reduce_max(
                        smax[:, 0:1], pe_[:], axis=mybir.AxisListType.X,
                        negate=True,
                    )
                    nc.scalar.activation(
                        exp_sb[:, it * L:(it + 1) * L], pe_[:], AF.Exp,
                        bias=smax[:, 0:1], accum_out=smax[:, 1:2],
                    )
                    nc.vector.reciprocal(smax[:, 2:3], smax[:, 1:2])
                    nc.vector.tensor_scalar_mul(
                        attn_r[:, it * L:(it + 1) * L],
                        exp_sb[:, it * L:(it + 1) * L],
                        smax[:, 2:3],
                    )
                # attn^T via PE transpose (f32r)
                for it in range(LT):
                    for jt in range(LT):
                        ptp = pstp.tile([128, 128], F32, tag="tp")
                        nc.tensor.transpose(
                            ptp[:].bitcast(F32R),
                            attn_r[:, it * L + jt * 128: it * L + (jt + 1) * 128],
                            idn_r,
                        )
                        nc.vector.tensor_copy(
                            attnt_r[:, jt * L + it * 128: jt * L + (it + 1) * 128],
                            ptp[:],
                        )

                # ---- betas^T / alphas^T (f32r) ----
                # ba cols: d (8) * 512 + [betasT 0:256 | alphasT 256:512]
                ba = onepool.tile([128, DT * 2 * L], F32R, tag="ba")
                for d in range(DT):
                    psb = ps512.tile([128, 2 * L], F32, tag="mm512")
                    for jt in range(LT):
                        nc.tensor.matmul(
                            psb[:, 0:L],
                            phseq_r[:, (2 + jt) * D + d * 128:
                                       (2 + jt) * D + (d + 1) * 128],
                            attnt_r[:, jt * L:(jt + 1) * L],
                            start=(jt == 0),
                            stop=(jt == LT - 1),
                        )
                    for it in range(LT):
                        nc.tensor.matmul(
                            psb[:, L:2 * L],
                            phseq_r[:, it * D + d * 128: it * D + (d + 1) * 128],
                            attn_r[:, it * L:(it + 1) * L],
                            start=(it == 0),
                            stop=(it == LT - 1),
                        )
                    nc.vector.tensor_copy(
                        ba[:, d * 2 * L:(d + 1) * 2 * L], psb[:]
                    )

                # ---- V stage (f32r): V = tanh([X|Y] @ W_G), sum-pooled ----
                for q in range(4):  # quarters of the FF output dim
                    psv = [ps512.tile([128, 2 * L], F32, tag="mm512", name=f"psv{q}_{_i}")
                           for _i in range(4)]
                    for k in range(KT):
                        wgt = spool.tile([128, 512], F32R, tag="wg")
                        nc.gpsimd.dma_start(
                            wgt[:],
                            w_g[k * 128:(k + 1) * 128,
                                q * 512:(q + 1) * 512],
                        )
                        if k < DT:
                            cat_k = pht_r[:, k * 2 * L:(k + 1) * 2 * L]
                        else:
                            cat_k = ba[:, (k - DT) * 2 * L:(k - DT + 1) * 2 * L]
                        for f4 in range(4):
                            nc.tensor.matmul(
                                psv[f4][:],
                                wgt[:, f4 * 128:(f4 + 1) * 128],
                                cat_k,
                                start=(k == 0),
                                stop=(k == KT - 1),
                            )
                    for f4 in range(4):
                        ff = q * 4 + f4
                        vs1 = scrpool.tile([128, L], F32, tag="vscr")
                        nc.scalar.activation(
                            vs1[:], psv[f4][:, 0:L], AF.Tanh,
                            accum_out=agg_sb[:, ff * B + b: ff * B + b + 1],
                        )
                        vs2 = scrpool.tile([128, L], F32, tag="vscr")
                        nc.scalar.activation(
                            vs2[:], psv[f4][:, L:2 * L], AF.Tanh,
                            accum_out=agg_sb[:, (FT_N + ff) * B + b:
                                               (FT_N + ff) * B + b + 1],
                        )

            # ---- collective 1: AllGather agg over batch ----
            ag_in = dpool.tile([128, 2 * FT_N * B], F32)
            ag_out = dpool.tile([N_CORES * 128, 2 * FT_N * B], F32)
            nc.sync.dma_start(ag_in[:], agg_sb[:])
            nc.gpsimd.collective_compute(
                "AllGather",
                mybir.AluOpType.bypass,
                replica_groups=[list(range(N_CORES))],
                ins=[ag_in.opt()],
                outs=[ag_out.opt()],
            )
            # readback: aggt cols r (32) * 32 + (rank*B + b)
            aggt = onepool.tile([128, 2 * FT_N * B_GLOBAL], F32, tag="aggt")
            ag_view = ag_out[:, :].rearrange(
                "(r p) (t b) -> p t r b", p=128, b=B
            )  # [128, 32, 8, 4]
            for t in range(2 * FT_N):
                nc.sync.dma_start(
                    aggt[:, t * B_GLOBAL:(t + 1) * B_GLOBAL].rearrange(
                        "p (r b) -> p r b", b=B
                    ),
                    ag_view[:, t, :, :],
                )

            # ---- MLP layer 1 (fp32): a1 = tanh(agg @ W1s + b1s) ----
            n_m = FF_SH // 128  # 2
            ps_a1 = [pstp.tile([128, B_GLOBAL], F32, tag="tp", name=f"psa1_{_i}") for _i in range(n_m)]
            for r in range(2 * FT_N):
                w1t = spool.tile([128, FF_SH], F32, tag="w1")
                nc.sync.dma_start(w1t[:], w1s[r * 128:(r + 1) * 128, :])
                for m in range(n_m):
                    nc.tensor.matmul(
                        ps_a1[m][:],
                        w1t[:, m * 128:(m + 1) * 128],
                        aggt[:, r * B_GLOBAL:(r + 1) * B_GLOBAL],
                        start=(r == 0),
                        stop=(r == 2 * FT_N - 1),
                    )
            a1t = onepool.tile([128, n_m * B_GLOBAL], F32, tag="a1t")
            for m in range(n_m):
                nc.scalar.activation(
                    a1t[:, m * B_GLOBAL:(m + 1) * B_GLOBAL], ps_a1[m][:],
                    AF.Tanh, bias=b1_sb[:, m:m + 1],
                )

            # ---- MLP layer 2 partial (fp32): z2p = a1 @ W2s ----
            z2p = onepool.tile([128, FT_N * B_GLOBAL], F32, tag="z2p")
            for q8 in range(FT_N // 2):
                ps_z = [pstp.tile([128, B_GLOBAL], F32, tag="tp", name=f"psz{q8}_{_i}")
                        for _i in range(2)]
                for r2 in range(n_m):
                    w2t = spool.tile([128, 256], F32, tag="w2")
                    nc.sync.dma_start(
                        w2t[:],
                        w2s[r2 * 128:(r2 + 1) * 128, q8 * 256:(q8 + 1) * 256],
                    )
                    for j in range(2):
                        nc.tensor.matmul(
                            ps_z[j][:],
                            w2t[:, j * 128:(j + 1) * 128],
                            a1t[:, r2 * B_GLOBAL:(r2 + 1) * B_GLOBAL],
                            start=(r2 == 0),
                            stop=(r2 == n_m - 1),
                        )
                for j in range(2):
                    t = q8 * 2 + j
                    nc.vector.tensor_copy(
                        z2p[:, t * B_GLOBAL:(t + 1) * B_GLOBAL], ps_z[j][:]
                    )

            # ---- collective 2: AllReduce z2 partials ----
            ar_in = dpool.tile([FF, B_GLOBAL], F32)
            ar_out = dpool.tile([FF, B_GLOBAL], F32)
            nc.sync.dma_start(
                ar_in[:, :].rearrange("(t p) b -> p t b", p=128),
                z2p[:].rearrange("p (t b) -> p t b", b=B_GLOBAL),
            )
            nc.gpsimd.collective_compute(
                "AllReduce",
                mybir.AluOpType.add,
                replica_groups=[list(range(N_CORES))],
                ins=[ar_in.opt()],
                outs=[ar_out.opt()],
            )

            # ---- a2 = tanh(z2 + b2); logits = a2 @ W3 + b3 ----
            a2t = onepool.tile([128, FT_N * B_GLOBAL], F32, tag="a2t")
            ps_lg = pstp.tile([128, B_GLOBAL], F32, tag="tp")
            for t in range(FT_N):
                z2f = scrpool.tile([128, B_GLOBAL], F32, tag="z2f")
                nc.sync.dma_start(
                    z2f[:], ar_out[t * 128:(t + 1) * 128, :]
                )
                nc.scalar.activation(
                    a2t[:, t * B_GLOBAL:(t + 1) * B_GLOBAL], z2f[:],
                    AF.Tanh, bias=b2_sb[:, t:t + 1],
                )
            for t in range(FT_N):
                nc.tensor.matmul(
                    ps_lg[:NCLS, :],
                    w3_sb[:, t, :],
                    a2t[:, t * B_GLOBAL:(t + 1) * B_GLOBAL],
                    start=(t == 0),
                    stop=(t == FT_N - 1),
                )
            lgt = scrpool.tile([128, B_GLOBAL], F32, tag="lgt")
            nc.vector.tensor_scalar_add(
                lgt[:NCLS, :], ps_lg[:NCLS, :], b3_sb[:NCLS, 0:1]
            )
            nc.sync.dma_start(out[:, :], lgt[:NCLS, :])

    nc.compile()
    return nc


# ---------------------------------------------------------------------------
# host-side runner (compiled once, reusable)
# ---------------------------------------------------------------------------

class _SpmdRunner:
    def __init__(self, nc, n_cores):
        import jax
        from jax.sharding import Mesh, PartitionSpec
        from jax.experimental.shard_map import shard_map
        from concourse.bass2jax import (
            _bass_exec_p, install_neuronx_cc_hook, partition_id_tensor,
        )

        install_neuronx_cc_hook()
        self.jax = jax
        self.n_cores = n_cores
        partition_name = (
            nc.partition_id_tensor.name if nc.partition_id_tensor else None
        )
        in_names, out_names, out_avals, zero_outs = [], [], [], []
        for alloc in nc.m.functions[0].allocations:
            if not isinstance(alloc, mybir.MemoryLocationSet):
                continue
            name = alloc.memorylocations[0].name
            if alloc.kind == "ExternalInput":
                if name != partition_name:
                    in_names.append(name)
            elif alloc.kind == "ExternalOutput":
                out_names.append(name)
                shape = tuple(alloc.tensor_shape)
                dtype = mybir.dt.np(alloc.dtype)
                out_avals.append(jax.core.ShapedArray(shape, dtype))
                zero_outs.append(np.zeros(shape, dtype))
        self.in_names = in_names
        self.out_names = out_names
        self.out_avals = out_avals
        self.zero_outs = zero_outs
        n_params = len(in_names)
        n_outs = len(out_avals)
        all_in_names = in_names + out_names
        if partition_name is not None:
            all_in_names.append(partition_name)
        donate = tuple(range(n_params, n_params + n_outs))

        def _body(*args):
            operands = list(args)
            if partition_name is not None:
                operands.append(partition_id_tensor())
            outs = _bass_exec_p.bind(
                *operands,
                out_avals=tuple(out_avals),
                in_names=tuple(all_in_names),
                out_names=tuple(out_names),
                lowering_input_output_aliases=(),
                sim_require_finite=True,
                sim_require_nnan=True,
                nc=nc,
            )
            return tuple(outs)

        devices = jax.devices()[:n_cores]
        assert len(devices) == n_cores
        mesh = Mesh(np.asarray(devices), ("core",))
        in_specs = (PartitionSpec("core"),) * (n_params + n_outs)
        out_specs = (PartitionSpec("core"),) * len(out_names)
        self._fn = jax.jit(
            shard_map(_body, mesh=mesh, in_specs=in_specs,
                      out_specs=out_specs, check_rep=False),
            donate_argnums=donate,
            keep_unused=True,
        )

    def __call__(self, in_maps):
        n_cores = self.n_cores
        per_core = [[np.asarray(m[k]) for k in self.in_names] for m in in_maps]
        concat_in = [
            np.concatenate([per_core[c][i] for c in range(n_cores)], axis=0)
            for i in range(len(self.in_names))
        ]
        concat_zeros = [
            np.zeros((n_cores * z.shape[0], *z.shape[1:]), z.dtype)
            for z in self.zero_outs
        ]
        out_arrs = self.jax.block_until_ready(self._fn(*concat_in, *concat_zeros))
        return [
            {
                name: np.asarray(out_arrs[i]).reshape(
                    n_cores, *self.out_avals[i].shape
                )[c]
                for i, name in enumerate(self.out_names)
            }
            for c in range(n_cores)
        ]


_RUNNER = None


def _get_runner():
    global _RUNNER
    if _RUNNER is None:
        nc = build()
        _RUNNER = _SpmdRunner(nc, N_CORES)
    return _RUNNER


def make_in_maps(premises, hypotheses, W_F, W_G, W1, b1, W2, b2, W3, b3):
    premises = np.ascontiguousarray(premises, np.float32)
    hypotheses = np.ascontiguousarray(hypotheses, np.float32)
    in_maps = []
    for k in range(N_CORES):
        sh = FF_SH
        in_maps.append({
            "premises": premises[k * B:(k + 1) * B],
            "hypotheses": hypotheses[k * B:(k + 1) * B],
            "W_F": np.ascontiguousarray(W_F, np.float32),
            "W_G": np.ascontiguousarray(W_G, np.float32),
            "W1s": np.ascontiguousarray(W1[:, k * sh:(k + 1) * sh], np.float32),
            "b1s": np.ascontiguousarray(
                b1[k * sh:(k + 1) * sh], np.float32).reshape(sh // 128, 128),
            "W2s": np.ascontiguousarray(W2[k * sh:(k + 1) * sh, :], np.float32),
            "b2r": np.ascontiguousarray(b2, np.float32).reshape(FF // 128, 128),
            "W3": np.ascontiguousarray(W3, np.float32),
            "b3": np.ascontiguousarray(b3, np.float32).reshape(NCLS, 1),
        })
    return in_maps


def kernel(premises, hypotheses, W_F, W_G, W1, b1, W2, b2, W3, b3):
    runner = _get_runner()
    in_maps = make_in_maps(
        premises, hypotheses, W_F, W_G, W1, b1, W2, b2, W3, b3
    )
    res = runner(in_maps)
    logits = np.ascontiguousarray(res[0]["logitsT"].T, np.float32)
    return logits


# revision 10
# speedup vs baseline: 23.6777x; 23.6777x over previous
"""Trainium2 Bass kernel for the Alignment (decomposable-attention) model.

Full inputs in, full outputs out.  Internally: data-parallel over batch
across 8 NeuronCores (4 batch elements per core) for the align/compare
phases; weight-sharded MLP classifier with AllGather(agg) + AllReduce(z2)
collectives.

Precision: the tanh-projection (F) stage runs in true fp32 matmuls (the
softmax selection is chaotic w.r.t. e-matrix errors); the e/attention/
compare stages run in float32r (~13-bit mantissa single-pass matmuls,
measured rel err ~1.5e-4 per dot), which end-to-end gives ~3e-3 relative
error on logits vs the fp32 reference.
"""
import sys

sys.path.insert(0, "/opt/trn_rl_repo")

import numpy as np

import concourse.bacc as bacc
import concourse.tile as tile
import concourse.mybir as mybir

F32 = mybir.dt.float32
F32R = mybir.dt.float32r
AF = mybir.ActivationFunctionType

N_CORES = 8
B_GLOBAL = 32
B = B_GLOBAL // N_CORES  # 4 local batch elements
L = 256
D = 1024
ALIGN = 1024
FF = 2048
FF_SH = FF // N_CORES  # 256: per-core shard of the MLP hidden cols/rows
NCLS = 3

DT = D // 128     # 8 k-tiles over D
AT = ALIGN // 128  # 8 m-tiles over ALIGN
LT = L // 128     # 2 tiles over sequence
KT = 2 * D // 128  # 16 contraction tiles over 2*D for W_G
FT_N = FF // 128   # 16 ff tiles


def build():
    nc = bacc.Bacc("TRN2", target_bir_lowering=False, debug=False,
                   num_devices=N_CORES)

    prem = nc.dram_tensor("premises", [B, L, D], F32, kind="ExternalInput")
    hypo = nc.dram_tensor("hypotheses", [B, L, D], F32, kind="ExternalInput")
    w_f = nc.dram_tensor("W_F", [D, ALIGN], F32, kind="ExternalInput")
    w_g = nc.dram_tensor("W_G", [2 * D, FF], F32, kind="ExternalInput")
    w1s = nc.dram_tensor("W1s", [2 * FF, FF_SH], F32, kind="ExternalInput")
    b1s = nc.dram_tensor("b1s", [FF_SH // 128, 128], F32, kind="ExternalInput")
    w2s = nc.dram_tensor("W2s", [FF_SH, FF], F32, kind="ExternalInput")
    b2r = nc.dram_tensor("b2r", [FF // 128, 128], F32, kind="ExternalInput")
    w3 = nc.dram_tensor("W3", [FF, NCLS], F32, kind="ExternalInput")
    b3 = nc.dram_tensor("b3", [NCLS, 1], F32, kind="ExternalInput")
    out = nc.dram_tensor("logitsT", [NCLS, B_GLOBAL], F32, kind="ExternalOutput")

    idn_dram = nc.inline_tensor(np.eye(128, dtype=np.float32), name="idn")

    with tile.TileContext(nc) as tc:
        with (
            tc.tile_pool(name="const", bufs=1) as cpool,
            tc.tile_pool(name="stream", bufs=3) as spool,
            tc.tile_pool(name="act", bufs=2) as apool,
            tc.tile_pool(name="single", bufs=1) as onepool,
            tc.tile_pool(name="scr", bufs=4) as scrpool,
            tc.tile_pool(name="ps512", bufs=4, space="PSUM") as ps512,
            tc.tile_pool(name="pse", bufs=2, space="PSUM") as pse,
            tc.tile_pool(name="pstp", bufs=2, space="PSUM") as pstp,
            tc.tile_pool(name="dram", bufs=1, space="DRAM") as dpool,
        ):
            # ---- constants ----
            wf_sb = cpool.tile([128, DT, ALIGN], F32, tag="wf")
            nc.sync.dma_start(
                wf_sb[:], w_f.ap().rearrange("(t p) a -> p t a", p=128)
            )
            idn = cpool.tile([128, 128], F32, tag="idn")
            nc.sync.dma_start(idn[:], idn_dram.ap())
            idn_r = idn[:].bitcast(F32R)
            b1_sb = cpool.tile([128, FF_SH // 128], F32, tag="b1")
            nc.sync.dma_start(b1_sb[:], b1s.ap().rearrange("m p -> p m"))
            b2_sb = cpool.tile([128, FF // 128], F32, tag="b2")
            nc.sync.dma_start(b2_sb[:], b2r.ap().rearrange("m p -> p m"))
            b3_sb = cpool.tile([128, 1], F32, tag="b3")
            nc.sync.dma_start(b3_sb[:NCLS, :], b3.ap())
            w3_sb = cpool.tile([128, FT_N, NCLS], F32, tag="w3")
            nc.sync.dma_start(
                w3_sb[:], w3.ap().rearrange("(t p) c -> p t c", p=128)
            )

            agg_sb = onepool.tile([128, 2 * FT_N * B], F32, tag="agg")

            for b in range(B):
                # ---- load this batch element's premise/hypothesis ----
                # phseq cols: [P_i0 | P_i1 | H_j0 | H_j1], each 1024 wide
                phseq = apool.tile([128, 4 * D], F32, tag="phseq")
                for it in range(LT):
                    nc.sync.dma_start(
                        phseq[:, it * D:(it + 1) * D],
                        prem[b, it * 128:(it + 1) * 128, :],
                    )
                    nc.sync.dma_start(
                        phseq[:, (LT + it) * D:(LT + it + 1) * D],
                        hypo[b, it * 128:(it + 1) * 128, :],
                    )
                # f32r copy for the attention-apply (betas/alphas) matmuls
                phseq_r = onepool.tile([128, 4 * D], F32R, tag="phseqr")
                nc.gpsimd.dma_start(
                    phseq_r[:, 0:2 * D].rearrange("p (t d) -> p t d", d=D),
                    prem[b].rearrange("(t p) d -> p t d", p=128),
                )
                nc.gpsimd.dma_start(
                    phseq_r[:, 2 * D:4 * D].rearrange("p (t d) -> p t d", d=D),
                    hypo[b].rearrange("(t p) d -> p t d", p=128),
                )

                # ---- transpose P, H into feature-major PHT ----
                # pht cols: d-tile d (8) * 512 + [P.T cols 0:256 | H.T 256:512]
                pht = apool.tile([128, DT * 2 * L], F32, tag="pht")
                for side in range(2):  # 0: P, 1: H
                    for it in range(LT):
                        src_col = (side * LT + it) * D
                        for d in range(DT):
                            ptp = pstp.tile([128, 128], F32, tag="tp")
                            nc.tensor.transpose(
                                ptp[:],
                                phseq[:, src_col + d * 128: src_col + (d + 1) * 128],
                                idn[:],
                            )
                            nc.vector.tensor_copy(
                                pht[:, d * 2 * L + side * L + it * 128:
                                       d * 2 * L + side * L + (it + 1) * 128],
                                ptp[:],
                            )
                # f32r copy of PHT for the compare (V) stage
                pht_r = onepool.tile([128, DT * 2 * L], F32R, tag="phtr")
                for d in range(DT):
                    nc.vector.tensor_copy(
                        pht_r[:, d * 2 * L:(d + 1) * 2 * L],
                        pht[:, d * 2 * L:(d + 1) * 2 * L],
                    )

                # ---- F stage (fp32): F = tanh([P|H] @ W_F), feature-major ----
                # ft cols: a-tile a (8) * 512 + [Fp 0:256 | Fh 256:512]
                ft = onepool.tile([128, AT * 2 * L], F32R, tag="ft")
                for a in range(AT):
                    psf = ps512.tile([128, 2 * L], F32, tag="mm512")
                    for d in range(DT):
                        nc.tensor.matmul(
                            psf[:],
                            wf_sb[:, d, a * 128:(a + 1) * 128],
                            pht[:, d * 2 * L:(d + 1) * 2 * L],
                            start=(d == 0),
                            stop=(d == DT - 1),
                        )
                    nc.scalar.activation(
                        ft[:, a * 2 * L:(a + 1) * 2 * L], psf[:], AF.Tanh
                    )

                # ---- e + softmax (f32r matmul, fp32 softmax) ----
                exp_sb = onepool.tile([128, LT * L], F32, tag="exp")
                attn_r = onepool.tile([128, LT * L], F32R, tag="attn")
                attnt_r = onepool.tile([128, LT * L], F32R, tag="attnt")
                for it in range(LT):
                    smax = scrpool.tile([128, 3], F32, tag="smax")
                    pe_ = pse.tile([128, L], F32, tag="e")
                    for a in range(AT):
                        nc.tensor.matmul(
                            pe_[:],
                            ft[:, a * 2 * L + it * 128: a * 2 * L + (it + 1) * 128],
                            ft[:, a * 2 * L + L: (a + 1) * 2 * L],
                            start=(a == 0),
                            stop=(a == AT - 1),
                        )
                    nc.vector.reduce_max(
                        smax[:, 0:1], pe_[:], axis=mybir.AxisListType.X,
                        negate=True,
                    )
                    nc.scalar.activation(
                        exp_sb[:, it * L:(it + 1) * L], pe_[:], AF.Exp,
                        bias=smax[:, 0:1], accum_out=smax[:, 1:2],
                    )
                    nc.vector.reciprocal(smax[:, 2:3], smax[:, 1:2])
                    nc.vector.tensor_scalar_mul(
                        attn_r[:, it * L:(it + 1) * L],
                        exp_sb[:, it * L:(it + 1) * L],
                        smax[:, 2:3],
                    )
                # attn^T via PE transpose (f32r)
                for it in range(LT):
                    for jt in range(LT):
                        ptp = pstp.tile([128, 128], F32, tag="tp")
                        nc.tensor.transpose(
                            ptp[:].bitcast(F32R),
                            attn_r[:, it * L + jt * 128: it * L + (jt + 1) * 128],
                            idn_r,
                        )
                        nc.vector.tensor_copy(
                            attnt_r[:, jt * L + it * 128: jt * L + (it + 1) * 128],
                            ptp[:],
                        )

                # ---- betas^T / alphas^T (f32r) ----
                # ba cols: d (8) * 512 + [betasT 0:256 | alphasT 256:512]
                ba = onepool.tile([128, DT * 2 * L], F32R, tag="ba")
                for d in range(DT):
                    psb = ps512.tile([128, 2 * L], F32, tag="mm512")
                    for jt in range(LT):
                        nc.tensor.matmul(
                            psb[:, 0:L],
                            phseq_r[:, (2 + jt) * D + d * 128:
                                       (2 + jt) * D + (d + 1) * 128],
                            attnt_r[:, jt * L:(jt + 1) * L],
                            start=(jt == 0),
                            stop=(jt == LT - 1),
                        )
                    for it in range(LT):
                        nc.tensor.matmul(
                            psb[:, L:2 * L],
                            phseq_r[:, it * D + d * 128: it * D + (d + 1) * 128],
                            attn_r[:, it * L:(it + 1) * L],
                            start=(it == 0),
                            stop=(it == LT - 1),
                        )
                    nc.vector.tensor_copy(
                        ba[:, d * 2 * L:(d + 1) * 2 * L], psb[:]
                    )

                # ---- V stage (f32r): V = tanh([X|Y] @ W_G), sum-pooled ----
                for q in range(4):  # quarters of the FF output dim
                    psv = [ps512.tile([128, 2 * L], F32, tag="mm512", name=f"psv{q}_{_i}")
                           for _i in range(4)]
                    for k in range(KT):
                        wgt = spool.tile([128, 512], F32R, tag="wg")
                        nc.gpsimd.dma_start(
                            wgt[:],
                            w_g[k * 128:(k + 1) * 128,
                                q * 512:(q + 1) * 512],
                        )
                        if k < DT:
                            cat_k = pht_r[:, k * 2 * L:(k + 1) * 2 * L]
                        else:
                            cat_k = ba[:, (k - DT) * 2 * L:(k - DT + 1) * 2 * L]
                        for f4 in range(4):
                            nc.tensor.matmul(
                                psv[f4][:],
                                wgt[:, f4 * 128:(f4 + 1) * 128],
                                cat_k,
                                start=(k == 0),
                                stop=(k == KT - 1),
                            )
                    for f4 in range(4):
                        ff = q * 4 + f4
                        vs1 = scrpool.tile([128, L], F32, tag="vscr")
                        nc.scalar.activation(
                            vs1[:], psv[f4][:, 0:L], AF.Tanh,
                            accum_out=agg_sb[:, ff * B + b: ff * B + b + 1],
                        )
                        vs2 = scrpool.tile([128, L], F32, tag="vscr")
                        nc.scalar.activation(
                            vs2[:], psv[f4][:, L:2 * L], AF.Tanh,
                            accum_out=agg_sb[:, (FT_N + ff) * B + b:
                                               (FT_N + ff) * B + b + 1],
                        )

            # ---- collective 1: AllGather agg over batch ----
            ag_in = dpool.tile([128, 2 * FT_N * B], F32)
            ag_out = dpool.tile([N_CORES * 128, 2 * FT_N * B], F32)
            nc.sync.dma_start(ag_in[:], agg_sb[:])
            nc.gpsimd.collective_compute(
                "AllGather",
                mybir.AluOpType.bypass,
                replica_groups=[list(range(N_CORES))],
                ins=[ag_in.opt()],
                outs=[ag_out.opt()],
            )
            # readback: aggt cols r (32) * 32 + (rank*B + b)
            aggt = onepool.tile([128, 2 * FT_N * B_GLOBAL], F32, tag="aggt")
            ag_view = ag_out[:, :].rearrange(
                "(r p) (t b) -> p t r b", p=128, b=B
            )  # [128, 32, 8, 4]
            for t in range(2 * FT_N):
                nc.sync.dma_start(
                    aggt[:, t * B_GLOBAL:(t + 1) * B_GLOBAL].rearrange(
                        "p (r b) -> p r b", b=B
                    ),
                    ag_view[:, t, :, :],
                )

            # ---- MLP layer 1 (fp32): a1 = tanh(agg @ W1s + b1s) ----
            n_m = FF_SH // 128  # 2
            ps_a1 = [pstp.tile([128, B_GLOBAL], F32, tag="tp", name=f"psa1_{_i}") for _i in range(n_m)]
            for r in range(2 * FT_N):
                w1t = spool.tile([128, FF_SH], F32, tag="w1")
                nc.sync.dma_start(w1t[:], w1s[r * 128:(r + 1) * 128, :])
                for m in range(n_m):
                    nc.tensor.matmul(
                        ps_a1[m][:],
                        w1t[:, m * 128:(m + 1) * 128],
                        aggt[:, r * B_GLOBAL:(r + 1) * B_GLOBAL],
                        start=(r == 0),
                        stop=(r == 2 * FT_N - 1),
                    )
            a1t = onepool.tile([128, n_m * B_GLOBAL], F32, tag="a1t")
            for m in range(n_m):
                nc.scalar.activation(
                    a1t[:, m * B_GLOBAL:(m + 1) * B_GLOBAL], ps_a1[m][:],
                    AF.Tanh, bias=b1_sb[:, m:m + 1],
                )

            # ---- MLP layer 2 partial (fp32): z2p = a1 @ W2s ----
            z2p = onepool.tile([128, FT_N * B_GLOBAL], F32, tag="z2p")
            for q8 in range(FT_N // 2):
                ps_z = [pstp.tile([128, B_GLOBAL], F32, tag="tp", name=f"psz{q8}_{_i}")
                        for _i in range(2)]
                for r2 in range(n_m):
                    w2t = spool.tile([128, 256], F32, tag="w2")
                    nc.sync.dma_start(
                        w2t[:],
                        w2s[r2 * 128:(r2 + 1) * 128, q8 * 256:(q8 + 1) * 256],
                    )
                    for j in range(2):
                        nc.tensor.matmul(
                            ps_z[j][:],
                            w2t[:, j * 128:(j + 1) * 128],
                            a1t[:, r2 * B_GLOBAL:(r2 + 1) * B_GLOBAL],
                            start=(r2 == 0),
                            stop=(r2 == n_m - 1),
                        )
                for j in range(2):
                    t = q8 * 2 + j
                    nc.vector.tensor_copy(
                        z2p[:, t * B_GLOBAL:(t + 1) * B_GLOBAL], ps_z[j][:]
                    )

            # ---- collective 2: AllReduce z2 partials ----
            ar_in = dpool.tile([FF, B_GLOBAL], F32)
            ar_out = dpool.tile([FF, B_GLOBAL], F32)
            nc.sync.dma_start(
                ar_in[:, :].rearrange("(t p) b -> p t b", p=128),
                z2p[:].rearrange("p (t b) -> p t b", b=B_GLOBAL),
            )
            nc.gpsimd.collective_compute(
                "AllReduce",
                mybir.AluOpType.add,
                replica_groups=[list(range(N_CORES))],
                ins=[ar_in.opt()],
                outs=[ar_out.opt()],
            )

            # ---- a2 = tanh(z2 + b2); logits = a2 @ W3 + b3 ----
            a2t = onepool.tile([128, FT_N * B_GLOBAL], F32, tag="a2t")
            ps_lg = pstp.tile([128, B_GLOBAL], F32, tag="tp")
            for t in range(FT_N):
                z2f = scrpool.tile([128, B_GLOBAL], F32, tag="z2f")
                nc.sync.dma_start(
                    z2f[:], ar_out[t * 128:(t + 1) * 128, :]
                )
                nc.scalar.activation(
                    a2t[:, t * B_GLOBAL:(t + 1) * B_GLOBAL], z2f[:],
                    AF.Tanh, bias=b2_sb[:, t:t + 1],
                )
            for t in range(FT_N):
                nc.tensor.matmul(
                    ps_lg[:NCLS, :],
                    w3_sb[:, t, :],
                    a2t[:, t * B_GLOBAL:(t + 1) * B_GLOBAL],
                    start=(t == 0),
                    stop=(t == FT_N - 1),
                )
            lgt = scrpool.tile([128, B_GLOBAL], F32, tag="lgt")
            nc.vector.tensor_scalar_add(
                lgt[:NCLS, :], ps_lg[:NCLS, :], b3_sb[:NCLS, 0:1]
            )
            nc.sync.dma_start(out[:, :], lgt[:NCLS, :])

    nc.compile()
    return nc


# ---------------------------------------------------------------------------
# host-side runner (compiled once, reusable)
# ---------------------------------------------------------------------------

class _SpmdRunner:
    def __init__(self, nc, n_cores):
        import jax
        from jax.sharding import Mesh, PartitionSpec
        from jax.experimental.shard_map import shard_map
        from concourse.bass2jax import (
            _bass_exec_p, install_neuronx_cc_hook, partition_id_tensor,
        )

        install_neuronx_cc_hook()
        self.jax = jax
        self.n_cores = n_cores
        partition_name = (
            nc.partition_id_tensor.name if nc.partition_id_tensor else None
        )
        in_names, out_names, out_avals, zero_outs = [], [], [], []
        for alloc in nc.m.functions[0].allocations:
            if not isinstance(alloc, mybir.MemoryLocationSet):
                continue
            name = alloc.memorylocations[0].name
            if alloc.kind == "ExternalInput":
                if name != partition_name:
                    in_names.append(name)
            elif alloc.kind == "ExternalOutput":
                out_names.append(name)
                shape = tuple(alloc.tensor_shape)
                dtype = mybir.dt.np(alloc.dtype)
                out_avals.append(jax.core.ShapedArray(shape, dtype))
                zero_outs.append(np.zeros(shape, dtype))
        self.in_names = in_names
        self.out_names = out_names
        self.out_avals = out_avals
        self.zero_outs = zero_outs
        n_params = len(in_names)
        n_outs = len(out_avals)
        all_in_names = in_names + out_names
        if partition_name is not None:
            all_in_names.append(partition_name)
        donate = tuple(range(n_params, n_params + n_outs))

        def _body(*args):
            operands = list(args)
            if partition_name is not None:
                operands.append(partition_id_tensor())
            outs = _bass_exec_p.bind(
                *operands,
                out_avals=tuple(out_avals),
                in_names=tuple(all_in_names),
                out_names=tuple(out_names),
                lowering_input_output_aliases=(),
                sim_require_finite=True,
                sim_require_nnan=True,
                nc=nc,
            )
            return tuple(outs)

        devices = jax.devices()[:n_cores]
        assert len(devices) == n_cores
        mesh = Mesh(np.asarray(devices), ("core",))
        self._mesh = mesh
        in_specs = (PartitionSpec("core"),) * (n_params + n_outs)
        out_specs = (PartitionSpec("core"),) * len(out_names)
        self._fn = jax.jit(
            shard_map(_body, mesh=mesh, in_specs=in_specs,
                      out_specs=out_specs, check_rep=False),
            donate_argnums=donate,
            keep_unused=True,
        )

    def prepare(self, in_maps):
        """Upload per-core inputs to the device mesh; returns device arrays."""
        n_cores = self.n_cores
        per_core = [[np.asarray(m[k]) for k in self.in_names] for m in in_maps]
        concat_in = [
            np.concatenate([per_core[c][i] for c in range(n_cores)], axis=0)
            for i in range(len(self.in_names))
        ]
        jax = self.jax
        from jax.sharding import NamedSharding, PartitionSpec
        shardings = [
            NamedSharding(self._mesh, PartitionSpec("core"))
            for _ in concat_in
        ]
        dev = [jax.device_put(a, s) for a, s in zip(concat_in, shardings)]
        return jax.block_until_ready(dev)

    def run_prepared(self, dev_in):
        n_cores = self.n_cores
        concat_zeros = [
            np.zeros((n_cores * z.shape[0], *z.shape[1:]), z.dtype)
            for z in self.zero_outs
        ]
        out_arrs = self.jax.block_until_ready(self._fn(*dev_in, *concat_zeros))
        return [
            {
                name: np.asarray(out_arrs[i]).reshape(
                    n_cores, *self.out_avals[i].shape
                )[c]
                for i, name in enumerate(self.out_names)
            }
            for c in range(n_cores)
        ]

    def __call__(self, in_maps):
        return self.run_prepared(self.prepare(in_maps))


_RUNNER = None


def _get_runner():
    global _RUNNER
    if _RUNNER is None:
        nc = build()
        _RUNNER = _SpmdRunner(nc, N_CORES)
    return _RUNNER


def make_in_maps(premises, hypotheses, W_F, W_G, W1, b1, W2, b2, W3, b3):
    premises = np.ascontiguousarray(premises, np.float32)
    hypotheses = np.ascontiguousarray(hypotheses, np.float32)
    in_maps = []
    for k in range(N_CORES):
        sh = FF_SH
        in_maps.append({
            "premises": premises[k * B:(k + 1) * B],
            "hypotheses": hypotheses[k * B:(k + 1) * B],
            "W_F": np.ascontiguousarray(W_F, np.float32),
            "W_G": np.ascontiguousarray(W_G, np.float32),
            "W1s": np.ascontiguousarray(W1[:, k * sh:(k + 1) * sh], np.float32),
            "b1s": np.ascontiguousarray(
                b1[k * sh:(k + 1) * sh], np.float32).reshape(sh // 128, 128),
            "W2s": np.ascontiguousarray(W2[k * sh:(k + 1) * sh, :], np.float32),
            "b2r": np.ascontiguousarray(b2, np.float32).reshape(FF // 128, 128),
            "W3": np.ascontiguousarray(W3, np.float32),
            "b3": np.ascontiguousarray(b3, np.float32).reshape(NCLS, 1),
        })
    return in_maps


def kernel(premises, hypotheses, W_F, W_G, W1, b1, W2, b2, W3, b3):
    runner = _get_runner()
    in_maps = make_in_maps(
        premises, hypotheses, W_F, W_G, W1, b1, W2, b2, W3, b3
    )
    res = runner(in_maps)
    logits = np.ascontiguousarray(res[0]["logitsT"].T, np.float32)
    return logits


# revision 19
# speedup vs baseline: 23.7862x; 1.0046x over previous
"""Trainium2 Bass kernel for the Alignment (decomposable-attention) model.

Full inputs in, full outputs out.  Internally: data-parallel over batch
across 8 NeuronCores (4 batch elements per core) for the align/compare
phases; weight-sharded MLP classifier with AllGather(agg) + AllReduce(z2)
collectives.

Precision: the tanh-projection (F) stage runs in true fp32 matmuls (the
softmax selection is chaotic w.r.t. e-matrix errors); the e/attention/
compare stages run in float32r (~13-bit mantissa single-pass matmuls,
measured rel err ~1.5e-4 per dot), which end-to-end gives ~3e-3 relative
error on logits vs the fp32 reference.
"""
import sys

sys.path.insert(0, "/opt/trn_rl_repo")

import numpy as np

import concourse.bacc as bacc
import concourse.tile as tile
import concourse.mybir as mybir

F32 = mybir.dt.float32
F32R = mybir.dt.float32r
BF16 = mybir.dt.bfloat16
AF = mybir.ActivationFunctionType

N_CORES = 8
B_GLOBAL = 32
B = B_GLOBAL // N_CORES  # 4 local batch elements
L = 256
D = 1024
ALIGN = 1024
FF = 2048
FF_SH = FF // N_CORES  # 256: per-core shard of the MLP hidden cols/rows
NCLS = 3

DT = D // 128     # 8 k-tiles over D
AT = ALIGN // 128  # 8 m-tiles over ALIGN
LT = L // 128     # 2 tiles over sequence
KT = 2 * D // 128  # 16 contraction tiles over 2*D for W_G
FT_N = FF // 128   # 16 ff tiles


def build(repl=1):
    """repl > 1 replicates the per-batch align/compare phase for timing
    calibration (the dispatch overhead of the axon tunnel is ~200ms, so
    kernel exec time is measured as the slope over repl)."""
    nc = bacc.Bacc("TRN2", target_bir_lowering=False, debug=False,
                   num_devices=N_CORES)

    prem = nc.dram_tensor("premises", [B, L, D], F32, kind="ExternalInput")
    hypo = nc.dram_tensor("hypotheses", [B, L, D], F32, kind="ExternalInput")
    w_fhi = nc.dram_tensor("W_F_hi", [D, ALIGN], BF16, kind="ExternalInput")
    w_flo = nc.dram_tensor("W_F_lo", [D, ALIGN], BF16, kind="ExternalInput")
    w_g = nc.dram_tensor("W_G", [2 * D, FF], F32, kind="ExternalInput")
    w1s = nc.dram_tensor("W1s", [2 * FF, FF_SH], F32, kind="ExternalInput")
    b1s = nc.dram_tensor("b1s", [FF_SH // 128, 128], F32, kind="ExternalInput")
    w2s = nc.dram_tensor("W2s", [FF_SH, FF], F32, kind="ExternalInput")
    b2r = nc.dram_tensor("b2r", [FF // 128, 128], F32, kind="ExternalInput")
    w3 = nc.dram_tensor("W3", [FF, NCLS], F32, kind="ExternalInput")
    b3 = nc.dram_tensor("b3", [NCLS, 1], F32, kind="ExternalInput")
    out = nc.dram_tensor("logitsT", [NCLS, B_GLOBAL], F32, kind="ExternalOutput")

    idn_dram = nc.inline_tensor(np.eye(128, dtype=np.float32), name="idn")

    with tile.TileContext(nc) as tc:
        with (
            tc.tile_pool(name="const", bufs=1) as cpool,
            tc.tile_pool(name="stream", bufs=3) as spool,
            tc.tile_pool(name="act", bufs=2) as apool,
            tc.tile_pool(name="single", bufs=1) as onepool,
            tc.tile_pool(name="scr", bufs=4) as scrpool,
            tc.tile_pool(name="ps512", bufs=4, space="PSUM") as ps512,
            tc.tile_pool(name="pse", bufs=2, space="PSUM") as pse,
            tc.tile_pool(name="pstp", bufs=2, space="PSUM") as pstp,
            tc.tile_pool(name="dram", bufs=1, space="DRAM") as dpool,
        ):
            # ---- constants ----
            wfhi_sb = cpool.tile([128, DT, ALIGN], BF16, tag="wfhi")
            nc.sync.dma_start(
                wfhi_sb[:], w_fhi.ap().rearrange("(t p) a -> p t a", p=128)
            )
            wflo_sb = cpool.tile([128, DT, ALIGN], BF16, tag="wflo")
            nc.sync.dma_start(
                wflo_sb[:], w_flo.ap().rearrange("(t p) a -> p t a", p=128)
            )
            idn = cpool.tile([128, 128], F32, tag="idn")
            nc.sync.dma_start(idn[:], idn_dram.ap())
            idn_r = idn[:].bitcast(F32R)
            b1_sb = cpool.tile([128, FF_SH // 128], F32, tag="b1")
            nc.sync.dma_start(b1_sb[:], b1s.ap().rearrange("m p -> p m"))
            b2_sb = cpool.tile([128, FF // 128], F32, tag="b2")
            nc.sync.dma_start(b2_sb[:], b2r.ap().rearrange("m p -> p m"))
            b3_sb = cpool.tile([128, 1], F32, tag="b3")
            nc.sync.dma_start(b3_sb[:NCLS, :], b3.ap())
            w3_sb = cpool.tile([128, FT_N, NCLS], F32, tag="w3")
            nc.sync.dma_start(
                w3_sb[:], w3.ap().rearrange("(t p) c -> p t c", p=128)
            )

            agg_sb = onepool.tile([128, 2 * FT_N * B], F32, tag="agg")

            for pair_iter in range(2 * repl):
                pr = pair_iter % 2
                bpair = (2 * pr, 2 * pr + 1)
                pht_hi, pht_lo, pht_r_d, phseq_r_d, ft_d = {}, {}, {}, {}, {}
                for b in bpair:
                    # f32r seq-major copy for the attention-apply matmuls
                    phseq_r = onepool.tile([128, 4 * D], F32R, tag="phseqr",
                                           name=f"phseqr_{b}")
                    nc.gpsimd.dma_start(
                        phseq_r[:, 0:2 * D].rearrange("p (t d) -> p t d", d=D),
                        prem[b].rearrange("(t p) d -> p t d", p=128),
                    )
                    nc.gpsimd.dma_start(
                        phseq_r[:, 2 * D:4 * D].rearrange("p (t d) -> p t d", d=D),
                        hypo[b].rearrange("(t p) d -> p t d", p=128),
                    )
                    phseq_r_d[b] = phseq_r

                    # ---- transpose P, H into feature-major split tiles ----
                    # cols: d-tile d (8) * 512 + [P.T cols 0:256 | H.T 256:512]
                    hi = apool.tile([128, DT * 2 * L], BF16, tag="phthi",
                                    name=f"phthi_{b}")
                    lo = apool.tile([128, DT * 2 * L], BF16, tag="phtlo",
                                    name=f"phtlo_{b}")
                    pr_ = apool.tile([128, DT * 2 * L], F32R, tag="phtr",
                                     name=f"phtr_{b}")
                    for side, src in ((0, prem), (1, hypo)):
                        for it in range(LT):
                            for d in range(DT):
                                tin = scrpool.tile([128, 128], F32, tag="tpin")
                                nc.sync.dma_start(
                                    tin[:],
                                    src[b, it * 128:(it + 1) * 128,
                                        d * 128:(d + 1) * 128],
                                )
                                ptp = pstp.tile([128, 128], F32, tag="tp")
                                nc.tensor.transpose(ptp[:], tin[:], idn[:])
                                dst = (d * 2 * L + side * L + it * 128,
                                       d * 2 * L + side * L + (it + 1) * 128)
                                nc.vector.tensor_copy(
                                    hi[:, dst[0]:dst[1]], ptp[:])
                                nc.vector.tensor_sub(
                                    lo[:, dst[0]:dst[1]], ptp[:],
                                    hi[:, dst[0]:dst[1]])
                                nc.vector.tensor_copy(
                                    pr_[:, dst[0]:dst[1]], ptp[:])
                    pht_hi[b], pht_lo[b], pht_r_d[b] = hi, lo, pr_

                # ---- F stage (bf16x3): F = tanh([P|H] @ W_F) ----
                # chained matmuls share the stationary operand across the
                # pair so walrus dedupes the weight loads
                for b in bpair:
                    ft_d[b] = onepool.tile([128, AT * 2 * L], F32R, tag="ft",
                                           name=f"ft_{b}", bufs=2)
                for a in range(AT):
                    psf = {b: ps512.tile([128, 2 * L], F32, tag="mm512",
                                         name=f"psf_{b}") for b in bpair}
                    for d in range(DT):
                        whi = wfhi_sb[:, d, a * 128:(a + 1) * 128]
                        wlo = wflo_sb[:, d, a * 128:(a + 1) * 128]
                        sl = slice(d * 2 * L, (d + 1) * 2 * L)
                        for b in bpair:
                            nc.tensor.matmul(
                                psf[b][:], whi, pht_hi[b][:, sl],
                                start=(d == 0), stop=False)
                        for b in bpair:
                            nc.tensor.matmul(
                                psf[b][:], whi, pht_lo[b][:, sl],
                                start=False, stop=False)
                        for b in bpair:
                            nc.tensor.matmul(
                                psf[b][:], wlo, pht_hi[b][:, sl],
                                start=False, stop=(d == DT - 1))
                    for b in bpair:
                        nc.scalar.activation(
                            ft_d[b][:, a * 2 * L:(a + 1) * 2 * L],
                            psf[b][:], AF.Tanh)

                # ---- per-batch attention + compare ----
                for b in bpair:
                    ft = ft_d[b]
                    phseq_r = phseq_r_d[b]
                    pht_r = pht_r_d[b]
                    # ---- e + softmax (f32r matmul, fp32 softmax) ----
                    exp_sb = onepool.tile([128, LT * L], F32, tag="exp",
                                          name=f"exp_{b}")
                    attn_r = onepool.tile([128, LT * L], F32R, tag="attn",
                                          name=f"attn_{b}")
                    attnt_r = onepool.tile([128, LT * L], F32R, tag="attnt",
                                           name=f"attnt_{b}")
                    for it in range(LT):
                        smax = scrpool.tile([128, 3], F32, tag="smax")
                        pe_ = pse.tile([128, L], F32, tag="e")
                        for a in range(AT):
                            nc.tensor.matmul(
                                pe_[:],
                                ft[:, a * 2 * L + it * 128: a * 2 * L + (it + 1) * 128],
                                ft[:, a * 2 * L + L: (a + 1) * 2 * L],
                                start=(a == 0),
                                stop=(a == AT - 1),
                            )
                        nc.vector.reduce_max(
                            smax[:, 0:1], pe_[:], axis=mybir.AxisListType.X,
                            negate=True,
                        )
                        nc.scalar.activation(
                            exp_sb[:, it * L:(it + 1) * L], pe_[:], AF.Exp,
                            bias=smax[:, 0:1], accum_out=smax[:, 1:2],
                        )
                        nc.vector.reciprocal(smax[:, 2:3], smax[:, 1:2])
                        nc.vector.tensor_scalar_mul(
                            attn_r[:, it * L:(it + 1) * L],
                            exp_sb[:, it * L:(it + 1) * L],
                            smax[:, 2:3],
                        )
                    # attn^T via PE transpose (f32r)
                    for it in range(LT):
                        for jt in range(LT):
                            ptp = pstp.tile([128, 128], F32, tag="tp")
                            nc.tensor.transpose(
                                ptp[:].bitcast(F32R),
                                attn_r[:, it * L + jt * 128: it * L + (jt + 1) * 128],
                                idn_r,
                            )
                            nc.vector.tensor_copy(
                                attnt_r[:, jt * L + it * 128: jt * L + (it + 1) * 128],
                                ptp[:],
                            )

                    # ---- betas^T / alphas^T (f32r) ----
                    # ba cols: d (8) * 512 + [betasT 0:256 | alphasT 256:512]
                    ba = onepool.tile([128, DT * 2 * L], F32R, tag="ba",
                                      name=f"ba_{b}")
                    for d in range(DT):
                        psb = ps512.tile([128, 2 * L], F32, tag="mm512")
                        for jt in range(LT):
                            nc.tensor.matmul(
                                psb[:, 0:L],
                                phseq_r[:, (2 + jt) * D + d * 128:
                                           (2 + jt) * D + (d + 1) * 128],
                                attnt_r[:, jt * L:(jt + 1) * L],
                                start=(jt == 0),
                                stop=(jt == LT - 1),
                            )
                        for it in range(LT):
                            nc.tensor.matmul(
                                psb[:, L:2 * L],
                                phseq_r[:, it * D + d * 128: it * D + (d + 1) * 128],
                                attn_r[:, it * L:(it + 1) * L],
                                start=(it == 0),
                                stop=(it == LT - 1),
                            )
                        nc.vector.tensor_copy(
                            ba[:, d * 2 * L:(d + 1) * 2 * L], psb[:]
                        )

                    # ---- V stage (f32r): V = tanh([X|Y] @ W_G), sum-pooled ----
                    for q in range(4):  # quarters of the FF output dim
                        psv = [ps512.tile([128, 2 * L], F32, tag="mm512",
                                          name=f"psv{q}_{_i}")
                               for _i in range(4)]
                        for k in range(KT):
                            wgt = spool.tile([128, 512], F32R, tag="wg")
                            nc.gpsimd.dma_start(
                                wgt[:],
                                w_g[k * 128:(k + 1) * 128,
                                    q * 512:(q + 1) * 512],
                            )
                            if k < DT:
                                cat_k = pht_r[:, k * 2 * L:(k + 1) * 2 * L]
                            else:
                                cat_k = ba[:, (k - DT) * 2 * L:(k - DT + 1) * 2 * L]
                            for f4 in range(4):
                                nc.tensor.matmul(
                                    psv[f4][:],
                                    wgt[:, f4 * 128:(f4 + 1) * 128],
                                    cat_k,
                                    start=(k == 0),
                                    stop=(k == KT - 1),
                                )
                        for f4 in range(4):
                            ff = q * 4 + f4
                            vs1 = scrpool.tile([128, L], F32, tag="vscr")
                            nc.scalar.activation(
                                vs1[:], psv[f4][:, 0:L], AF.Tanh,
                                accum_out=agg_sb[:, ff * B + b: ff * B + b + 1],
                            )
                            vs2 = scrpool.tile([128, L], F32, tag="vscr")
                            nc.scalar.activation(
                                vs2[:], psv[f4][:, L:2 * L], AF.Tanh,
                                accum_out=agg_sb[:, (FT_N + ff) * B + b:
                                                   (FT_N + ff) * B + b + 1],
                            )

            # ---- collective 1: AllGather agg over batch ----
            ag_in = dpool.tile([128, 2 * FT_N * B], F32)
            ag_out = dpool.tile([N_CORES * 128, 2 * FT_N * B], F32)
            nc.sync.dma_start(ag_in[:], agg_sb[:])
            nc.gpsimd.collective_compute(
                "AllGather",
                mybir.AluOpType.bypass,
                replica_groups=[list(range(N_CORES))],
                ins=[ag_in.opt()],
                outs=[ag_out.opt()],
            )
            # readback: aggt cols r (32) * 32 + (rank*B + b)
            aggt = onepool.tile([128, 2 * FT_N * B_GLOBAL], F32, tag="aggt")
            ag_view = ag_out[:, :].rearrange(
                "(r p) (t b) -> p t r b", p=128, b=B
            )  # [128, 32, 8, 4]
            for t in range(2 * FT_N):
                nc.sync.dma_start(
                    aggt[:, t * B_GLOBAL:(t + 1) * B_GLOBAL].rearrange(
                        "p (r b) -> p r b", b=B
                    ),
                    ag_view[:, t, :, :],
                )

            # ---- MLP layer 1 (fp32): a1 = tanh(agg @ W1s + b1s) ----
            n_m = FF_SH // 128  # 2
            ps_a1 = [pstp.tile([128, B_GLOBAL], F32, tag="tp", name=f"psa1_{_i}") for _i in range(n_m)]
            for r in range(2 * FT_N):
                w1t = spool.tile([128, FF_SH], F32, tag="w1")
                nc.sync.dma_start(w1t[:], w1s[r * 128:(r + 1) * 128, :])
                for m in range(n_m):
                    nc.tensor.matmul(
                        ps_a1[m][:],
                        w1t[:, m * 128:(m + 1) * 128],
                        aggt[:, r * B_GLOBAL:(r + 1) * B_GLOBAL],
                        start=(r == 0),
                        stop=(r == 2 * FT_N - 1),
                    )
            a1t = onepool.tile([128, n_m * B_GLOBAL], F32, tag="a1t")
            for m in range(n_m):
                nc.scalar.activation(
                    a1t[:, m * B_GLOBAL:(m + 1) * B_GLOBAL], ps_a1[m][:],
                    AF.Tanh, bias=b1_sb[:, m:m + 1],
                )

            # ---- MLP layer 2 partial (fp32): z2p = a1 @ W2s ----
            z2p = onepool.tile([128, FT_N * B_GLOBAL], F32, tag="z2p")
            for q8 in range(FT_N // 2):
                ps_z = [pstp.tile([128, B_GLOBAL], F32, tag="tp", name=f"psz{q8}_{_i}")
                        for _i in range(2)]
                for r2 in range(n_m):
                    w2t = spool.tile([128, 256], F32, tag="w2")
                    nc.sync.dma_start(
                        w2t[:],
                        w2s[r2 * 128:(r2 + 1) * 128, q8 * 256:(q8 + 1) * 256],
                    )
                    for j in range(2):
                        nc.tensor.matmul(
                            ps_z[j][:],
                            w2t[:, j * 128:(j + 1) * 128],
                            a1t[:, r2 * B_GLOBAL:(r2 + 1) * B_GLOBAL],
                            start=(r2 == 0),
                            stop=(r2 == n_m - 1),
                        )
                for j in range(2):
                    t = q8 * 2 + j
                    nc.vector.tensor_copy(
                        z2p[:, t * B_GLOBAL:(t + 1) * B_GLOBAL], ps_z[j][:]
                    )

            # ---- collective 2: AllReduce z2 partials ----
            ar_in = dpool.tile([FF, B_GLOBAL], F32)
            ar_out = dpool.tile([FF, B_GLOBAL], F32)
            nc.sync.dma_start(
                ar_in[:, :].rearrange("(t p) b -> p t b", p=128),
                z2p[:].rearrange("p (t b) -> p t b", b=B_GLOBAL),
            )
            nc.gpsimd.collective_compute(
                "AllReduce",
                mybir.AluOpType.add,
                replica_groups=[list(range(N_CORES))],
                ins=[ar_in.opt()],
                outs=[ar_out.opt()],
            )

            # ---- a2 = tanh(z2 + b2); logits = a2 @ W3 + b3 ----
            a2t = onepool.tile([128, FT_N * B_GLOBAL], F32, tag="a2t")
            ps_lg = pstp.tile([128, B_GLOBAL], F32, tag="tp")
            for t in range(FT_N):
                z2f = scrpool.tile([128, B_GLOBAL], F32, tag="z2f")
                nc.sync.dma_start(
                    z2f[:], ar_out[t * 128:(t + 1) * 128, :]
                )
                nc.scalar.activation(
                    a2t[:, t * B_GLOBAL:(t + 1) * B_GLOBAL], z2f[:],
                    AF.Tanh, bias=b2_sb[:, t:t + 1],
                )
            for t in range(FT_N):
                nc.tensor.matmul(
                    ps_lg[:NCLS, :],
                    w3_sb[:, t, :],
                    a2t[:, t * B_GLOBAL:(t + 1) * B_GLOBAL],
                    start=(t == 0),
                    stop=(t == FT_N - 1),
                )
            lgt = scrpool.tile([128, B_GLOBAL], F32, tag="lgt")
            nc.vector.tensor_scalar_add(
                lgt[:NCLS, :], ps_lg[:NCLS, :], b3_sb[:NCLS, 0:1]
            )
            nc.sync.dma_start(out[:, :], lgt[:NCLS, :])

    nc.compile()
    return nc


# ---------------------------------------------------------------------------
# host-side runner (compiled once, reusable)
# ---------------------------------------------------------------------------

class _SpmdRunner:
    def __init__(self, nc, n_cores):
        import jax
        from jax.sharding import Mesh, PartitionSpec
        from jax.experimental.shard_map import shard_map
        from concourse.bass2jax import (
            _bass_exec_p, install_neuronx_cc_hook, partition_id_tensor,
        )

        install_neuronx_cc_hook()
        self.jax = jax
        self.n_cores = n_cores
        partition_name = (
            nc.partition_id_tensor.name if nc.partition_id_tensor else None
        )
        in_names, out_names, out_avals, zero_outs = [], [], [], []
        for alloc in nc.m.functions[0].allocations:
            if not isinstance(alloc, mybir.MemoryLocationSet):
                continue
            name = alloc.memorylocations[0].name
            if alloc.kind == "ExternalInput":
                if name != partition_name:
                    in_names.append(name)
            elif alloc.kind == "ExternalOutput":
                out_names.append(name)
                shape = tuple(alloc.tensor_shape)
                dtype = mybir.dt.np(alloc.dtype)
                out_avals.append(jax.core.ShapedArray(shape, dtype))
                zero_outs.append(np.zeros(shape, dtype))
        self.in_names = in_names
        self.out_names = out_names
        self.out_avals = out_avals
        self.zero_outs = zero_outs
        n_params = len(in_names)
        n_outs = len(out_avals)
        all_in_names = in_names + out_names
        if partition_name is not None:
            all_in_names.append(partition_name)
        donate = tuple(range(n_params, n_params + n_outs))

        def _body(*args):
            operands = list(args)
            if partition_name is not None:
                operands.append(partition_id_tensor())
            outs = _bass_exec_p.bind(
                *operands,
                out_avals=tuple(out_avals),
                in_names=tuple(all_in_names),
                out_names=tuple(out_names),
                lowering_input_output_aliases=(),
                sim_require_finite=True,
                sim_require_nnan=True,
                nc=nc,
            )
            return tuple(outs)

        devices = jax.devices()[:n_cores]
        assert len(devices) == n_cores
        mesh = Mesh(np.asarray(devices), ("core",))
        self._mesh = mesh
        in_specs = (PartitionSpec("core"),) * (n_params + n_outs)
        out_specs = (PartitionSpec("core"),) * len(out_names)
        self._fn = jax.jit(
            shard_map(_body, mesh=mesh, in_specs=in_specs,
                      out_specs=out_specs, check_rep=False),
            donate_argnums=donate,
            keep_unused=True,
        )

    def prepare(self, in_maps):
        """Upload per-core inputs to the device mesh; returns device arrays."""
        n_cores = self.n_cores
        per_core = [[np.asarray(m[k]) for k in self.in_names] for m in in_maps]
        concat_in = [
            np.concatenate([per_core[c][i] for c in range(n_cores)], axis=0)
            for i in range(len(self.in_names))
        ]
        jax = self.jax
        from jax.sharding import NamedSharding, PartitionSpec
        shardings = [
            NamedSharding(self._mesh, PartitionSpec("core"))
            for _ in concat_in
        ]
        dev = [jax.device_put(a, s) for a, s in zip(concat_in, shardings)]
        return jax.block_until_ready(dev)

    def run_prepared(self, dev_in):
        n_cores = self.n_cores
        concat_zeros = [
            np.zeros((n_cores * z.shape[0], *z.shape[1:]), z.dtype)
            for z in self.zero_outs
        ]
        out_arrs = self.jax.block_until_ready(self._fn(*dev_in, *concat_zeros))
        return [
            {
                name: np.asarray(out_arrs[i]).reshape(
                    n_cores, *self.out_avals[i].shape
                )[c]
                for i, name in enumerate(self.out_names)
            }
            for c in range(n_cores)
        ]

    def __call__(self, in_maps):
        return self.run_prepared(self.prepare(in_maps))


_RUNNER = None


def _get_runner():
    global _RUNNER
    if _RUNNER is None:
        nc = build()
        _RUNNER = _SpmdRunner(nc, N_CORES)
    return _RUNNER


def make_in_maps(premises, hypotheses, W_F, W_G, W1, b1, W2, b2, W3, b3):
    import ml_dtypes
    premises = np.ascontiguousarray(premises, np.float32)
    hypotheses = np.ascontiguousarray(hypotheses, np.float32)
    wf32 = np.asarray(W_F, np.float32)
    wf_hi = wf32.astype(ml_dtypes.bfloat16)
    wf_lo = (wf32 - wf_hi.astype(np.float32)).astype(ml_dtypes.bfloat16)
    in_maps = []
    for k in range(N_CORES):
        sh = FF_SH
        in_maps.append({
            "premises": premises[k * B:(k + 1) * B],
            "hypotheses": hypotheses[k * B:(k + 1) * B],
            "W_F_hi": wf_hi,
            "W_F_lo": wf_lo,
            "W_G": np.ascontiguousarray(W_G, np.float32),
            "W1s": np.ascontiguousarray(W1[:, k * sh:(k + 1) * sh], np.float32),
            "b1s": np.ascontiguousarray(
                b1[k * sh:(k + 1) * sh], np.float32).reshape(sh // 128, 128),
            "W2s": np.ascontiguousarray(W2[k * sh:(k + 1) * sh, :], np.float32),
            "b2r": np.ascontiguousarray(b2, np.float32).reshape(FF // 128, 128),
            "W3": np.ascontiguousarray(W3, np.float32),
            "b3": np.ascontiguousarray(b3, np.float32).reshape(NCLS, 1),
        })
    return in_maps


def kernel(premises, hypotheses, W_F, W_G, W1, b1, W2, b2, W3, b3):
    runner = _get_runner()
    in_maps = make_in_maps(
        premises, hypotheses, W_F, W_G, W1, b1, W2, b2, W3, b3
    )
    res = runner(in_maps)
    logits = np.ascontiguousarray(res[0]["logitsT"].T, np.float32)
    return logits
